# revision 28
# baseline (speedup 1.0000x reference)
"""Hex-masked sparse conv (ConvHex) as a Bass/Tile kernel on 8 TRN2 NeuronCores.

Strategy (v2)
-------------
Data-parallel over batch: 16 images -> 2 per core.

The conv has 19 hex taps in a 9x5 window (all with even dh+dw), C_in=64,
C_out=128. Active outputs all have even h+w parity, and they only ever read
even-parity input cells. So the host packs x to the even lattice
xe[c, r, k] with w = 2k + r%2 (67 slots/row) -- halving input bytes and
making every matmul rhs access unit-stride.

On device, per image, xe lives in SBUF as a [128, 209*67] bf16 tile:
partitions 0:64 hold xe, partitions 64:128 hold xe shifted down 4 rows.
This packs PAIRS of taps (dh, dw) and (dh+4, dw) into single K=128
matmuls (8 pairs + 3 singleton K=64 taps = 11 matmul streams). The tile
is filled by 3 sub-load DMAs per half so matmuls start ~5us into the
kernel instead of waiting for the full image.

Output rows are grouped into variable-height same-parity groups whose
[nrows, ncols] compact lattice fills a PSUM bank (<=512 f32). Per group:
11 accumulating matmuls (groups-outer order so each group drains while
the next computes), then elu(z)+1 = min(exp(z),1) + relu(z) via two
ScalarE ACTs and one VectorE STT writing bf16 into a per-(image,parity)
staging tile. One big DMA per (image,parity) on the GpSimd queue writes
staging to DRAM; the host scatters back to NHWC and subtracts the 1.
"""

import numpy as np
import ml_dtypes

# ---------------------------------------------------------------- constants
R = 2
CIN, COUT = 64, 128
H, W = 209, 133
OH, OW = H - 4 * R, W - 2 * R   # 201, 129
NBATCH, NCORES = 16, 8
NPER = NBATCH // NCORES         # 2 images per core
SL = 67                         # packed slots per input row
HW67 = H * SL                   # 14003
DH_SHIFT = 4                    # pair taps (dh, dw) with (dh+4, dw)
SHIFT_ROWS = DH_SHIFT           # shifted half offset, in input rows
PSUM_CAP = 512                  # f32 per PSUM bank per partition
MAX_NR = 16
PADF = 2 * SL * MAX_NR          # free-dim padding so row-slab APs stay in bounds

BF16 = ml_dtypes.bfloat16


def _hex_indices(radius):
    moves = np.array([[1, 1], [2, 0], [1, -1], [-1, -1], [-2, 0], [-1, 1]])
    out = [[2 * radius, radius]]
    for il in range(1, radius + 1):
        s = np.array([[2 * radius - 2 * il, radius]])
        cur = moves.repeat(il, axis=0).cumsum(axis=0)
        out.extend((s + cur).tolist())
    return np.array(out, dtype=np.int32)


def _make_out_mask():
    mr = (OW - 1) // 2
    f = np.zeros((mr * 4 + 1, mr * 2 + 1), dtype=np.float32)
    for ind in _hex_indices(mr):
        f[tuple(ind)] = 1.0
    i_cut = (mr * 4 + 1 - OH) // 2
    return f[i_cut:-i_cut, :]    # [OH, OW]


_TAPS = _hex_indices(R)          # [19, 2] (dh, dw), reference tap order j
_NTAPS = len(_TAPS)
assert all((dh + dw) % 2 == 0 for dh, dw in _TAPS)
_MASK = _make_out_mask()         # [201, 129] float32


def _make_streams():
    """Pair taps (dh,dw) with (dh+4,dw). Returns (pairs, singles) as tap indices."""
    idx = {tuple(t): j for j, t in enumerate(_TAPS.tolist())}
    used = set()
    pairs, singles = [], []
    for t in sorted(idx):
        if t in used or (t[0] - DH_SHIFT, t[1]) in idx:
            continue
        chain = [t]
        cur = t
        while (cur[0] + DH_SHIFT, cur[1]) in idx:
            cur = (cur[0] + DH_SHIFT, cur[1])
            chain.append(cur)
        for k in range(0, len(chain) - 1, 2):
            pairs.append((idx[chain[k]], idx[chain[k + 1]]))
            used.update(chain[k:k + 2])
        if len(chain) % 2:
            singles.append(idx[chain[-1]])
            used.add(chain[-1])
    assert len(pairs) * 2 + len(singles) == _NTAPS
    return pairs, singles


_PAIRS, _SINGLES = _make_streams()   # 8 pairs + 3 singles
_NSTREAMS = len(_PAIRS) + len(_SINGLES)


def _slot_shift(dh, dw, p):
    """Packed-slot offset of tap (dh,dw) for an output row of parity p."""
    if dh % 2 == 0:
        return dw // 2
    return (dw - 1) // 2 + p


def _make_groups():
    """Variable-height same-parity row groups filling a PSUM bank.

    Returns [(p, h0, nrows, k0, ncols, stg_off)] sorted by h0, where the
    group covers output rows h0, h0+2, ..., h0+2*(nrows-1) and compact
    slots k0..k0+ncols-1 (slot k of a parity-p row <-> w = p + 2*k), and
    stg_off is the group's column offset in its parity staging tile.
    """
    spans = []
    for h in range(OH):
        w_act = np.nonzero(_MASK[h])[0]
        spans.append((int(w_act[0]), int(w_act[-1])))
    groups = []
    stg_off = {0: 0, 1: 0}
    for p in (0, 1):
        rows = list(range(p, OH, 2))
        i = 0
        while i < len(rows):
            nr = 1
            lo, hi = spans[rows[i]]
            while i + nr < len(rows) and nr < MAX_NR:
                l2 = min(lo, spans[rows[i + nr]][0])
                h2 = max(hi, spans[rows[i + nr]][1])
                if (nr + 1) * ((h2 - l2) // 2 + 1) > PSUM_CAP:
                    break
                lo, hi, nr = l2, h2, nr + 1
            ncols = (hi - lo) // 2 + 1
            groups.append((p, rows[i], nr, (lo - p) // 2, ncols, stg_off[p]))
            stg_off[p] += nr * ncols
            i += nr
    groups.sort(key=lambda g: g[1])
    return groups, stg_off[0], stg_off[1]


_GROUPS, _STG_E, _STG_O = _make_groups()
_STG_PAD = (max(_STG_E, _STG_O) + 63) // 64 * 64

# x sub-load local row ranges (straight half; shifted half sources rows +4).
# Image 0's first chunk is split across the scalar and sync HWDGE queues so
# both halves land ~12us into the kernel; weights ride the gpsimd queue.
_SUBLOADS = [(0, 70), (70, 140), (140, H)]
_N_WARM_MM = 12                 # dummy matmuls to lift the HAM clock gate

# singleton taps run as K=64 row-tiled matmuls: one on the straight half of
# the PE array (rows 0:64), two on the shifted half (rows 64:128), where tap
# (dh, dw) reads as (dh-4, dw). The straight single runs concurrently with
# the first shifted one; the shifted pair accumulates into a second PSUM
# bank that the epilogue merges back in.
_S_LOW = (4, 0)
_S_UPS = [(8, 2), (4, 4)]


# ---------------------------------------------------------------- device program
_PROGRAM = None


def _build_program():
    import concourse.mybir as mybir
    from concourse import bacc
    from concourse.tile import TileContext

    f32 = mybir.dt.float32
    bf16 = mybir.dt.bfloat16
    Alu = mybir.AluOpType
    Act = mybir.ActivationFunctionType

    nc = bacc.Bacc("TRN2", target_bir_lowering=False, debug=False)
    xe_in = nc.declare_dram_parameter("xe", [NPER, CIN, H, SL], bf16, isOutput=False)
    wp_in = nc.declare_dram_parameter("wp", [128, len(_PAIRS) * COUT], bf16, isOutput=False)
    wq_in = nc.declare_dram_parameter("wq", [128, 2 * COUT], bf16, isOutput=False)
    bias_in = nc.declare_dram_parameter("bias", [COUT, 1], f32, isOutput=False)
    out_p = nc.declare_dram_parameter("out", [NPER, 2, COUT, _STG_PAD], bf16, isOutput=True)

    # per-parity output chunking: after the last group of each run, DMA the
    # staging column range the run covers. The final run is kept small so
    # the post-compute tail is short.
    par_groups = {p: [g for g in _GROUPS if g[0] == p] for p in (0, 1)}
    out_chunk_after = {}            # group -> (col_lo, col_hi)
    for p in (0, 1):
        gs = par_groups[p]
        n = len(gs)
        bounds = [0, n // 2, n - 1, n]
        for b0, b1 in zip(bounds[:-1], bounds[1:]):
            run = gs[b0:b1]
            lo = run[0][5]
            hi = run[-1][5] + run[-1][2] * run[-1][4]
            out_chunk_after[run[-1]] = (lo, hi)

    with TileContext(nc) as tc:
        with (
            tc.tile_pool(name="const", bufs=1) as cpool,
            tc.tile_pool(name="x", bufs=2) as xpool,
            tc.tile_pool(name="ps", bufs=6, space="PSUM") as pspool,
            tc.tile_pool(name="ps2", bufs=2, space="PSUM") as pspool2,
            tc.tile_pool(name="zp", bufs=4) as zpool,
            tc.tile_pool(name="ep", bufs=4) as epool,
            tc.tile_pool(name="rp", bufs=4) as rpool,
            tc.tile_pool(name="sp", bufs=4) as spool,
        ):
            xts = [xpool.tile([128, HW67 + PADF], bf16, name=f"xt{i}", tag="xt")
                   for i in range(NPER)]
            wp_t = cpool.tile([128, len(_PAIRS) * COUT], bf16)
            wq_t = cpool.tile([128, 2 * COUT], bf16)
            bias_t = cpool.tile([COUT, 1], f32)
            # head schedule: both HWDGE queues carry what the first matmuls
            # need. Descriptor COUNT (not bytes) dominates these transfers,
            # so the first chunk is split lo/hi between the queues and each
            # queue's first batch is exactly the first-group critical set.
            # scalar queue: straight rows 0:36, then single weights + bias
            nc.scalar.dma_start(
                xts[0][0:CIN, 0:36 * SL], xe_in[0, :, 0:36, :])
            nc.scalar.dma_start(wq_t[:], wq_in[:])
            nc.scalar.dma_start(bias_t[:], bias_in[:])
            # sync queue: pair weights, shifted rows 0:24, the rest of
            # chunk 0, then the remaining sub-loads below
            nc.sync.dma_start(wp_t[:], wp_in[:])
            nc.sync.dma_start(
                xts[0][CIN:128, 0:24 * SL],
                xe_in[0, :, SHIFT_ROWS:24 + SHIFT_ROWS, :])
            nc.sync.dma_start(
                xts[0][0:CIN, 36 * SL:70 * SL], xe_in[0, :, 36:70, :])
            nc.sync.dma_start(
                xts[0][CIN:128, 24 * SL:70 * SL],
                xe_in[0, :, 24 + SHIFT_ROWS:70 + SHIFT_ROWS, :])
            # warmup activations: preload the ACT function tables
            warm_t = cpool.tile([1, 1], f32)
            nc.scalar.activation(warm_t[0:1, 0:1], bias_t[0:1, 0:1], Act.Exp)
            nc.scalar.activation(warm_t[0:1, 0:1], bias_t[0:1, 0:1], Act.Relu)

            # dummy matmuls on a zeroed scratch tile: keeps the PE busy
            # during the x load so the HAM clock gate lifts before the real
            # matmuls begin
            if _N_WARM_MM:
                junk_t = cpool.tile([128, PSUM_CAP], bf16)
                nc.vector.memset(junk_t[:], 0.0)
                warm_ps = pspool.tile([128, PSUM_CAP], f32, name="psb", tag="psb")
                for _ in range(_N_WARM_MM):
                    nc.tensor.matmul(warm_ps[:], junk_t[:, 0:128], junk_t[:],
                                     start=True, stop=True)

            for n in range(NPER):
                xt = xts[n]
                subloads = _SUBLOADS[1:] if n == 0 else _SUBLOADS
                for (l0, l1) in subloads:
                    nc.sync.dma_start(
                        xt[0:CIN, l0 * SL:l1 * SL], xe_in[n, :, l0:l1, :])
                    s0, s1 = l0 + SHIFT_ROWS, min(l1 + SHIFT_ROWS, H)
                    nc.sync.dma_start(
                        xt[CIN:128, l0 * SL:(l0 + s1 - s0) * SL],
                        xe_in[n, :, s0:s1, :])

                stg = {
                    0: spool.tile([128, _STG_PAD], bf16, name="se", tag="se"),
                    1: spool.tile([128, _STG_PAD], bf16, name="so", tag="so"),
                }

                def rhs_ap(h0, nr, k0, nc_, p, dh, dw, base, kpart):
                    o0 = (h0 + dh - base) * SL + k0 + _slot_shift(dh, dw, p)
                    p0 = 0 if base == 0 else CIN
                    sl_ = xt[p0:p0 + kpart, o0:o0 + 2 * SL * nr]
                    return sl_.rearrange("p (h q) -> p h q", h=nr)[:, :, 0:nc_]

                def finish(pend):
                    """Previous group's final STT + any output-chunk DMA.
                    Emitted one group late so the DVE queue never blocks on
                    this group's ACTs."""
                    (g, nf, e_t, a_t) = pend
                    (p, _h0, _nr, _k0, _nc, off) = g
                    # elu(z)+1 = min(exp(z),1) + relu(z); host subtracts 1
                    nc.vector.scalar_tensor_tensor(
                        stg[p][:, off:off + nf], e_t[:, 0:nf], 1.0,
                        a_t[:, 0:nf], op0=Alu.min, op1=Alu.add)
                    chunk = out_chunk_after.get(g)
                    if chunk is not None:
                        lo, hi = chunk
                        # the very last two chunks (parities 0 and 1 of the
                        # final image) would serialize on the gpsimd queue;
                        # route the parity-0 one to the by-then-idle sync
                        # queue so both final transfers run in parallel
                        final_p0 = (n == NPER - 1 and p == 0
                                    and hi == _STG_E)
                        eng = nc.sync if final_p0 else nc.gpsimd
                        eng.dma_start(
                            out_p[n, p, :, lo:hi], stg[p][:, lo:hi])

                state = {"pending": None}

                def epilogue(g, gi, pt, pt2, nf):
                    # z = pt + bias + pt2 in two passes (an instruction may
                    # read only ONE non-scalar input from PSUM). The
                    # u = pt2 copy alternates between ACT and DVE to balance
                    # the two engines; z lands on DVE either way.
                    u_t = zpool.tile([128, PSUM_CAP], f32, name="ut", tag="ut")
                    if gi % 2 == 0:
                        nc.scalar.activation(u_t[:, 0:nf], pt2[:, 0:nf],
                                             Act.Copy)
                    else:
                        nc.vector.tensor_scalar(
                            u_t[:, 0:nf], pt2[:, 0:nf], 0.0, None,
                            op0=Alu.add)
                    z_t = zpool.tile([128, PSUM_CAP], f32, name="zt", tag="zt")
                    nc.vector.scalar_tensor_tensor(
                        z_t[:, 0:nf], pt[:, 0:nf], bias_t[:, 0:1],
                        u_t[:, 0:nf], op0=Alu.add, op1=Alu.add)
                    e_t = epool.tile([128, PSUM_CAP], f32)
                    a_t = rpool.tile([128, PSUM_CAP], f32, name="at", tag="at")
                    nc.scalar.activation(e_t[:, 0:nf], z_t[:, 0:nf], Act.Exp)
                    nc.scalar.activation(a_t[:, 0:nf], z_t[:, 0:nf], Act.Relu)
                    if state["pending"] is not None:
                        finish(state["pending"])
                    state["pending"] = (g, nf, e_t, a_t)

                def mk_views(g):
                    (p, h0, nr, k0, nc_, off) = g
                    nf = nr * nc_
                    pt = pspool.tile([128, PSUM_CAP], f32, name="psb", tag="psb")
                    pt2 = pspool2.tile([128, PSUM_CAP], f32, name="psc", tag="psc")
                    pv = pt[:, 0:nf].rearrange("p (h w) -> p h w", h=nr)
                    pv2 = pt2[:, 0:nf].rearrange("p (h w) -> p h w", h=nr)
                    return pt, pt2, pv, pv2, nf

                def pairs_mm(g, pv):
                    (p, h0, nr, k0, nc_, off) = g
                    for si, (i1, _i2) in enumerate(_PAIRS):
                        dh, dw = map(int, _TAPS[i1])
                        nc.tensor.matmul(
                            pv, wp_t[:, si * COUT:(si + 1) * COUT],
                            rhs_ap(h0, nr, k0, nc_, p, dh, dw, 0, 128),
                            start=(si == 0), stop=False)

                def single_mm(g, pv, half, tap, start, stop):
                    (p, h0, nr, k0, nc_, off) = g
                    dh, dw = tap
                    col = 0 if tap != (4, 4) else COUT
                    base = SHIFT_ROWS if half else 0
                    p0 = CIN if half else 0
                    nc.tensor.matmul(
                        pv, wq_t[p0:p0 + CIN, col:col + COUT],
                        rhs_ap(h0, nr, k0, nc_, p, dh, dw, base, CIN),
                        start=start, stop=stop)

                # Groups are processed in pairs (gA, gB) so that every PE
                # slot carries two taps: the lone second upper single of gA
                # runs concurrently with a lower single of gB that was
                # reassigned to gB's SECOND accumulator (pt2B):
                #   pairsA x8, [U(8,2)A || L(4,0)A], [U(4,4)A || L(4,4)B],
                #   pairsB x8, [U(8,2)B || L(4,0)B]
                # = 19 slots per 2 groups: the 9.5-pass floor.
                assert len(_GROUPS) % 2 == 0
                for ai in range(0, len(_GROUPS), 2):
                    gA, gB = _GROUPS[ai], _GROUPS[ai + 1]
                    ptA, pt2A, pvA, pv2A, nfA = mk_views(gA)
                    pairs_mm(gA, pvA)
                    single_mm(gA, pv2A, True, _S_UPS[0], True, False)
                    single_mm(gA, pvA, False, _S_LOW, False, True)
                    ptB, pt2B, pvB, pv2B, nfB = mk_views(gB)
                    single_mm(gA, pv2A, True, _S_UPS[1], False, True)
                    single_mm(gB, pv2B, False, _S_UPS[1], True, False)
                    epilogue(gA, ai, ptA, pt2A, nfA)
                    pairs_mm(gB, pvB)
                    single_mm(gB, pv2B, True, _S_UPS[0], False, True)
                    single_mm(gB, pvB, False, _S_LOW, False, True)
                    epilogue(gB, ai + 1, ptB, pt2B, nfB)
                finish(state["pending"])
                state["pending"] = None
    nc.compile()
    return nc


def _get_program():
    global _PROGRAM
    if _PROGRAM is None:
        _PROGRAM = _build_program()
    return _PROGRAM


# ---------------------------------------------------------------- host wrapper
def _prep_inputs(x, sparse_weights, offset):
    x = np.asarray(x, np.float32)
    xt = np.ascontiguousarray(x.transpose(0, 3, 1, 2))       # [16, 64, H, W]
    xe = np.zeros((NBATCH, CIN, H, SL), BF16)
    xe[:, :, 0::2, :] = xt[:, :, 0::2, 0::2]
    xe[:, :, 1::2, 0:SL - 1] = xt[:, :, 1::2, 1::2]
    sw3 = np.asarray(sparse_weights, np.float32).reshape(CIN, COUT, _NTAPS)
    wp = np.empty((128, len(_PAIRS) * COUT), np.float32)
    for s, (i1, i2) in enumerate(_PAIRS):
        wp[0:CIN, s * COUT:(s + 1) * COUT] = sw3[:, :, i1]
        wp[CIN:128, s * COUT:(s + 1) * COUT] = sw3[:, :, i2]
    coord = {tuple(t): j for j, t in enumerate(_TAPS.tolist())}
    wq = np.zeros((128, 2 * COUT), np.float32)
    wq[0:CIN, 0:COUT] = sw3[:, :, coord[_S_LOW]]
    wq[CIN:128, 0:COUT] = sw3[:, :, coord[tuple(_S_UPS[0])]]
    wq[CIN:128, COUT:2 * COUT] = sw3[:, :, coord[tuple(_S_UPS[1])]]
    wq[0:CIN, COUT:2 * COUT] = sw3[:, :, coord[tuple(_S_UPS[1])]]
    bias = np.asarray(offset, np.float32).reshape(COUT, 1)
    return xe, wp.astype(BF16), wq.astype(BF16), bias


def kernel(x, sparse_weights, offset):
    from concourse.bass_utils import run_bass_kernel_spmd

    xe, wp, wq, bias = _prep_inputs(x, sparse_weights, offset)
    nc = _get_program()
    in_maps = [
        {"xe": xe[c * NPER:(c + 1) * NPER], "wp": wp, "wq": wq, "bias": bias}
        for c in range(NCORES)
    ]
    res = run_bass_kernel_spmd(nc, in_maps, list(range(NCORES)))
    arr = np.concatenate([np.asarray(res.results[c]["out"])
                          for c in range(NCORES)], axis=0)  # [16,2,128,STG] bf16
    arr = arr.astype(np.float32)
    full = np.zeros((NBATCH, OH, OW, COUT), np.float32)
    for (p, h0, nr, k0, nc_, off) in _GROUPS:
        blk = arr[:, p, :, off:off + nr * nc_].reshape(NBATCH, COUT, nr, nc_)
        full[:, h0:h0 + 2 * nr:2, p + 2 * k0:p + 2 * (k0 + nc_):2, :] = (
            blk.transpose(0, 2, 3, 1) - 1.0)
    full[:, _MASK == 0] = 0.0
    return full
